# revision 27
# baseline (speedup 1.0000x reference)
"""Trainium2 Bass kernel for nn_DetectUDPModel (rank-2 Hermitian detection loss).

Math: the reference computes
    loss = sum_m |v_m|^2,   v = B @ vec(matH),  B = (basis_re - i*basis_im).reshape(m, n*n)
where matH = lam0 * evc0 evc0^H - lam1 * evc1 evc1^H is rank-2.  Therefore
    v_m = lam0 * u0^T B_m conj(u0) - lam1 * u1^T B_m conj(u1)
with u_j = evc_j and B_m = basis_re[m] - i*basis_im[m].  Writing u = ur + i*ui,
each bilinear form over a real matrix X in {R_m, I_m} reduces to the four
scalars s[x,y] = x^T X y with x,y in {ur, ui}:
    u^T X conj(u) = [ur^T X ur + ui^T X ui] + i*[ui^T X ur - ur^T X ui]
    F = u^T R conj(u) - i * (u^T I conj(u))

Device work (memory-bound streaming of the basis):
  stage 1 on the TensorEngine with U = [ur0 ui0 ur1 ui1] (128, 4) STATIONARY
  and the basis matrices as the fp16 moving operand (1 cycle/row; fp32 would
  be 4):   out[x, j*128+b] = sum_a U[a, x] * X_j[a, b]   (4 matrices/matmul)
  The basis is cast to fp16 and pre-transposed on the host so each chunk DMA
  is a fully contiguous HBM read landing partition-outer in SBUF; this also
  halves the DMA bytes.  Measured end-to-end loss rel-err of the fp16
  quantization is ~2e-5 (errors average down ~sqrt(2048) across m).
Tiny stage 2 (contract T over b with exact f64 U) + final combine on host.

m is sharded across the 8 NeuronCores; per-core partial T tensors are
gathered and reduced on host (equivalent to the scalar all-reduce).
"""

import numpy as np

M_TOTAL = 2048
N = 128
N_CORES = 8
M_LOCAL = M_TOTAL // N_CORES   # 256 matrices per input tensor per core
STREAM = 2 * M_LOCAL           # 512 matrices per core (re then im)

# Device pipeline shape
CHUNK = 16        # matrices per input DMA (16 * 32 KiB fp16 = 0.5 MiB)
MATS_PER_MM = 4   # 4*128 = 512 moving columns per matmul (one PSUM bank)
GROUP_MATS = 64   # matrices per PSUM bank tile / out-DMA group (= PB)
IN_DMA_ENGINES = ("sync", "scalar")  # HWDGE rings for input DMAs (round-robin)
OUT_DMA_ENGINES = ("gpsimd",)        # engines for output DMAs (round-robin)
BT_BUFS = 6                          # input tile buffering depth per ring
PSUM_BUFS = 6
COPY_MODE = "vector"                 # "vector": copy PSUM->SBUF then DMA;
                                     # "none": DMA straight from PSUM

_CACHE: dict = {}


def _build_nc(stream_mats: int, chunk: int, group_mats: int,
              copy_mode: str = None):
    """Build + compile the per-core SPMD program. Returns the compiled Bacc.

    Per group of `group_mats` matrices, group_mats/4 matmuls accumulate into
    one (group_mats, 512) PSUM bank tile: matmul i uses a (128, group_mats)
    fp16 weight plane that is zero except columns 4i..4i+4 = U, so its T rows
    land on partitions 4i..4i+4 and the zero rows accumulate nothing.
    """
    import concourse.bacc as bacc
    import concourse.mybir as mybir
    from concourse import tile

    if copy_mode is None:
        copy_mode = COPY_MODE
    f16 = mybir.dt.float16
    f32 = mybir.dt.float32
    n_chunks = stream_mats // chunk
    n_groups = stream_mats // group_mats
    assert stream_mats % chunk == 0 and stream_mats % group_mats == 0
    assert chunk % MATS_PER_MM == 0 and group_mats % chunk == 0
    assert MATS_PER_MM * N * 4 <= 2048  # out rows fit one PSUM bank
    mm_per_group = group_mats // MATS_PER_MM

    nc = bacc.Bacc("TRN2", target_bir_lowering=False, debug=False,
                   num_devices=N_CORES)
    # xs[c, a, mi*N + b] = stream-matrix (c*chunk + mi) element [a, b]
    xs_in = nc.dram_tensor("xs", [n_chunks, N, chunk * N], f16,
                           kind="ExternalInput")
    # u[a, i*group_mats + col]: weight plane for matmul-slot i of a group
    u_in = nc.dram_tensor("u", [N, mm_per_group * group_mats], f16,
                          kind="ExternalInput")
    t_out = nc.dram_tensor("t_out", [n_groups, group_mats, MATS_PER_MM * N],
                           f32, kind="ExternalOutput")

    with tile.TileContext(nc) as tc:
        with (
            tc.tile_pool(name="bt0", bufs=BT_BUFS) as bpool0,
            tc.tile_pool(name="bt1", bufs=BT_BUFS) as bpool1,
            tc.tile_pool(name="ps", bufs=PSUM_BUFS, space="PSUM") as ppool,
            tc.tile_pool(name="st", bufs=2) as spool,
            tc.tile_pool(name="cn", bufs=1) as cpool,
        ):
            in_engines = [getattr(nc, e) for e in IN_DMA_ENGINES]
            bpools = [bpool0, bpool1]
            out_engines = [getattr(nc, e) for e in OUT_DMA_ENGINES]
            u_t = cpool.tile([N, mm_per_group * group_mats], f16)
            nc.gpsimd.dma_start(u_t[:], u_in[:])
            mm_per_chunk = chunk // MATS_PER_MM
            for g in range(n_groups):
                psum = ppool.tile([group_mats, MATS_PER_MM * N], f32)
                for ci, c in enumerate(range(g * group_mats // chunk,
                                             (g + 1) * group_mats // chunk)):
                    ring = c % len(in_engines)
                    bt = bpools[ring % 2].tile([N, chunk * N], f16)
                    in_engines[ring].dma_start(bt[:], xs_in[c])
                    for q in range(mm_per_chunk):
                        i = ci * mm_per_chunk + q
                        nc.tensor.matmul(
                            psum[:],
                            u_t[:, i * group_mats:(i + 1) * group_mats],
                            bt[:, q * MATS_PER_MM * N:(q + 1) * MATS_PER_MM * N],
                            start=(i == 0),
                            stop=(i == mm_per_group - 1),
                        )
                if copy_mode == "vector":
                    stage = spool.tile([group_mats, MATS_PER_MM * N], f32)
                    nc.vector.tensor_copy(stage[:], psum[:])
                    src = stage
                else:
                    src = psum
                out_engines[g % len(out_engines)].dma_start(t_out[g], src[:])
    nc.compile()
    return nc


def _get_nc():
    key = (STREAM, CHUNK, GROUP_MATS, COPY_MODE)
    if key not in _CACHE:
        _CACHE[key] = _build_nc(*key)
    return _CACHE[key]


def _host_prep(theta: np.ndarray, evl: np.ndarray):
    """Eigenvector/eigenvalue prep (tiny, f64 on host)."""
    theta = np.asarray(theta, dtype=np.float64)
    evl = np.asarray(evl, dtype=np.float64)
    c0 = theta[0] + 1j * theta[1]
    evc0 = c0 / np.linalg.norm(c0)
    c1 = theta[2] + 1j * theta[3]
    c1 = c1 - np.vdot(evc0, c1) * evc0
    evc1 = c1 / np.linalg.norm(c1)
    lam = np.log1p(np.exp(evl))
    lam = lam / np.linalg.norm(lam)
    U = np.stack([evc0.real, evc0.imag, evc1.real, evc1.imag], axis=1)
    return U, lam  # f64 (128, 4), f64 (2,)


def _pack_stream(basis_re_k: np.ndarray, basis_im_k: np.ndarray) -> np.ndarray:
    """fp16-cast + chunk-transpose one core's slice to the xs layout."""
    stream = np.concatenate([basis_re_k, basis_im_k], axis=0).astype(np.float16)
    n_chunks = stream.shape[0] // CHUNK
    # (t, a, b) -> (c, a, mi*N + b)
    xs = stream.reshape(n_chunks, CHUNK, N, N).transpose(0, 2, 1, 3)
    return np.ascontiguousarray(xs.reshape(n_chunks, N, CHUNK * N))


def _decode(t_raw: np.ndarray, U: np.ndarray, lam: np.ndarray) -> float:
    """Host stage 2 + combine for one core's t_out. Returns partial loss."""
    n_groups = t_raw.shape[0]
    # t_raw[g, 4*i + x, j*128 + b] -> T_all[t, b, x], t = g*GROUP_MATS + i*4 + j
    T_all = np.transpose(
        t_raw.reshape(n_groups, GROUP_MATS // 4, 4, MATS_PER_MM, N),
        (0, 1, 3, 4, 2)
    ).reshape(n_groups * GROUP_MATS, N, 4).astype(np.float64)
    TR, TI = T_all[:M_LOCAL], T_all[M_LOCAL:]
    sR = np.einsum('mbx,by->mxy', TR, U)
    sI = np.einsum('mbx,by->mxy', TI, U)
    v = np.zeros(M_LOCAL, dtype=np.complex128)
    for j, sgn in ((0, 1.0), (1, -1.0)):
        r0, i0 = 2 * j, 2 * j + 1
        F_re = sR[:, r0, r0] + sR[:, i0, i0] + sI[:, i0, r0] - sI[:, r0, i0]
        F_im = sR[:, i0, r0] - sR[:, r0, i0] - sI[:, r0, r0] - sI[:, i0, i0]
        v += sgn * lam[j] * (F_re + 1j * F_im)
    return float(np.sum(v.real ** 2 + v.imag ** 2))


def _make_in_maps(basis_re, basis_im, theta, evl):
    U, lam = _host_prep(theta, evl)
    u16 = U.astype(np.float16)
    # Shifted weight planes: plane i is zero except cols 4i..4i+4 = U
    mm_per_group = GROUP_MATS // MATS_PER_MM
    planes = np.zeros((N, mm_per_group, GROUP_MATS), dtype=np.float16)
    for i in range(mm_per_group):
        planes[:, i, 4 * i:4 * i + 4] = u16
    u_packed = np.ascontiguousarray(planes.reshape(N, mm_per_group * GROUP_MATS))
    basis_re = np.asarray(basis_re, dtype=np.float32)
    basis_im = np.asarray(basis_im, dtype=np.float32)
    in_maps = []
    for k in range(N_CORES):
        sl = slice(k * M_LOCAL, (k + 1) * M_LOCAL)
        in_maps.append({
            "xs": _pack_stream(basis_re[sl], basis_im[sl]),
            "u": u_packed,
        })
    return in_maps, U, lam


def _run_device(in_maps, **kwargs):
    from concourse.bass_utils import run_bass_kernel_spmd
    nc = _get_nc()
    return run_bass_kernel_spmd(nc, in_maps, list(range(N_CORES)), **kwargs)


def kernel(basis_re, basis_im, theta, evl) -> np.ndarray:
    in_maps, U, lam = _make_in_maps(basis_re, basis_im, theta, evl)
    res = _run_device(in_maps)
    total = 0.0
    for k in range(N_CORES):
        total += _decode(res.results[k]["t_out"], U, lam)
    return np.float32(total)
